# revision 12
# baseline (speedup 1.0000x reference)
"""KNN-impute kernel (nn_CalcImpute) for Trainium2, 8 NeuronCores.

Computation (see reference): for each of 8192 receiver rows, find the 16
smallest entries of a 50000-wide distance row (ties -> lowest column index,
matching jax.lax.top_k), gather fit_X_col at those columns, and output the
mean of the valid (mask==0) donor values (0 if none valid).

Sharding: pure data parallel over rows; each of the 8 cores gets 1024 rows.
fit/mask-derived tables are tiny and replicated.

Device algorithm per 128-row tile (rows live in partitions):
  P1  stream the 50000 columns in panels, segmented min (seg=50) ->
      1000 segment minima per row.  (the only full pass over the data)
  P2  negate seg-mins; 4 rounds of max8/max_index/match_replace give the
      24 segments with the smallest minima per row (+ the 25th min for a
      coverage flag).  All top-16 elements provably live in the 16 segs
      with smallest minima, so 24 gives slack.
  P3  indirect-DMA gather of those 24 segments (24x50 f32) per row from
      DRAM, plus the matching [G; V] table slices (G = fitX * valid,
      V = valid, precomputed on host).
  P4  negate candidates; 2x(max8+match_replace) marks the 16 smallest
      (ties by scan position); selection mask = (orig != replaced);
      numerator/denominator via fused multiply-reduce; res = num/den
      (den==0 -> den=1).  A 3rd max8 gives the 17th value for the
      tie-ambiguity flag.

Exactness: match_replace/max_index replace/report first occurrences, so the
selected *set* equals jax.lax.top_k's except when the 16th and 17th smallest
values are exactly equal (~0.3% of rows) or segment coverage is ambiguous.
Those rows are flagged on device and recomputed exactly on host.
"""

import os
import sys

for _p in ("/opt/trn_rl_repo", "/root/.axon_site/_ro/trn_rl_repo"):
    if os.path.isdir(_p) and _p not in sys.path:
        sys.path.insert(0, _p)

import numpy as np

import concourse.bass as bass
import concourse.bacc as bacc_mod
import concourse.mybir as mybir
import concourse.tile as tile
from concourse.bass_utils import run_bass_kernel_spmd

N_CORES = 8
R_TOTAL = 8192
N = 50000
P = 128              # SBUF partitions
S = 50               # segment size for the min prefilter
NSEG = N // S        # 1000 segments per row
PC = 12500           # panel columns streamed per DMA
NPAN = N // PC       # 4 panels
NSEG_P = PC // S     # 250 segments per panel
KSEG = 24            # candidate segments gathered per row
CAND = KSEG * S      # 1200 candidate values per row
NEG_BIG = -3.0e38    # replacement sentinel on the negated scale
F32 = mybir.dt.float32
U32 = mybir.dt.uint32


def build_bass(rows: int):
    """Bass program for one core processing `rows` rows (multiple of 128)."""
    assert rows % P == 0
    nt = rows // P

    nc = bacc_mod.Bacc()
    dist = nc.dram_tensor("dist", [rows, N], F32, kind="ExternalInput")
    gv = nc.dram_tensor("gv", [NSEG, 2 * S], F32, kind="ExternalInput")
    out_res = nc.dram_tensor("res", [P, nt], F32, kind="ExternalOutput")
    out_flag = nc.dram_tensor("flag", [P, nt], F32, kind="ExternalOutput")

    # flat views for indirect gathers (offset must be 0)
    dist_flat = dist[:, :].rearrange("r (s e) -> (r s) e", e=S)
    gv_flat = gv[:, :]

    with tile.TileContext(nc) as tc:
        with (
            tc.tile_pool(name="panels", bufs=2) as pan_pool,
            tc.tile_pool(name="segs", bufs=2) as seg_pool,
            tc.tile_pool(name="small", bufs=2) as small_pool,
            tc.tile_pool(name="cands", bufs=2) as cand_pool,
            tc.tile_pool(name="persist", bufs=1) as persist_pool,
        ):
            res_sb = persist_pool.tile([P, nt], F32)
            flag_sb = persist_pool.tile([P, nt], F32)

            for rt in range(nt):
                # ---- P1: stream panels, segmented min ----
                segmin = seg_pool.tile([P, NSEG], F32, tag="segmin")
                for pan in range(NPAN):
                    x = pan_pool.tile([P, PC], F32, tag="panel")
                    nc.gpsimd.dma_start(
                        out=x,
                        in_=dist[rt * P:(rt + 1) * P, pan * PC:(pan + 1) * PC],
                    )
                    nc.vector.tensor_reduce(
                        out=segmin[:, pan * NSEG_P:(pan + 1) * NSEG_P],
                        in_=x.rearrange("p (s e) -> p s e", e=S),
                        axis=mybir.AxisListType.X,
                        op=mybir.AluOpType.min,
                    )

                # ---- P2: 24 smallest seg-mins + their segment indices ----
                nsm_a = seg_pool.tile([P, NSEG], F32, tag="nsm_a")
                nsm_b = seg_pool.tile([P, NSEG], F32, tag="nsm_b")
                nc.vector.tensor_scalar_mul(nsm_a, segmin, -1.0)
                segidx = small_pool.tile([P, KSEG], U32, tag="segidx")
                v_seg = small_pool.tile([P, 4, 8], F32, tag="v_seg")
                cur, nxt = nsm_a, nsm_b
                for rnd in range(3):
                    v8 = v_seg[:, rnd, :]
                    nc.vector.max(out=v8, in_=cur)
                    nc.vector.max_index(
                        out=segidx[:, rnd * 8:(rnd + 1) * 8],
                        in_max=v8, in_values=cur)
                    nc.vector.match_replace(
                        out=nxt, in_to_replace=v8, in_values=cur,
                        imm_value=NEG_BIG)
                    cur, nxt = nxt, cur
                # 25th smallest seg-min (negated) for the coverage flag
                nc.vector.max(out=v_seg[:, 3, :], in_=cur)

                # ---- P3: gather candidate segments + G/V slices ----
                # per-partition segment base for this row-tile:
                # (rt*128 + p) * NSEG
                rowbase = small_pool.tile([P, 1], U32, tag="rowbase")
                nc.gpsimd.iota(rowbase, pattern=[[0, 1]], base=rt * P * NSEG,
                               channel_multiplier=NSEG)
                off_dist = small_pool.tile([P, KSEG], U32, tag="off_dist")
                nc.vector.tensor_tensor(
                    out=off_dist, in0=segidx,
                    in1=rowbase.to_broadcast([P, KSEG]),
                    op=mybir.AluOpType.add)
                # HW SWDGE indirect gather is only reliable with one offset
                # per partition, so issue one gather per candidate column.
                cand = cand_pool.tile([P, KSEG, S], F32, tag="cand")
                gvc = cand_pool.tile([P, KSEG, 2 * S], F32, tag="gvc")
                for t in range(KSEG):
                    nc.gpsimd.indirect_dma_start(
                        out=cand[:, t, :], out_offset=None,
                        in_=dist_flat,
                        in_offset=bass.IndirectOffsetOnAxis(
                            ap=off_dist[:, t:t + 1], axis=0),
                    )
                    nc.gpsimd.indirect_dma_start(
                        out=gvc[:, t, :], out_offset=None,
                        in_=gv_flat,
                        in_offset=bass.IndirectOffsetOnAxis(
                            ap=segidx[:, t:t + 1], axis=0),
                    )

                # ---- P4: exact top-16 + weighted mean ----
                ncand = cand_pool.tile([P, CAND], F32, tag="ncand")
                ncand2 = cand_pool.tile([P, CAND], F32, tag="ncand2")
                ncand3 = cand_pool.tile([P, CAND], F32, tag="ncand3")
                nc.vector.tensor_scalar_mul(
                    ncand, cand.rearrange("p a b -> p (a b)"), -1.0)
                v_c = small_pool.tile([P, 3, 8], F32, tag="v_c")
                nc.vector.max(out=v_c[:, 0, :], in_=ncand)
                nc.vector.match_replace(
                    out=ncand2, in_to_replace=v_c[:, 0, :], in_values=ncand,
                    imm_value=NEG_BIG)
                nc.vector.max(out=v_c[:, 1, :], in_=ncand2)
                nc.vector.match_replace(
                    out=ncand3, in_to_replace=v_c[:, 1, :], in_values=ncand2,
                    imm_value=NEG_BIG)
                nc.vector.max(out=v_c[:, 2, :], in_=ncand3)

                sel = cand_pool.tile([P, CAND], F32, tag="sel")
                nc.vector.tensor_tensor(
                    out=sel, in0=ncand, in1=ncand3,
                    op=mybir.AluOpType.not_equal)

                junk = cand_pool.tile([P, CAND], F32, tag="junk")
                acc = small_pool.tile([P, 8], F32, tag="acc")
                num, den = acc[:, 0:1], acc[:, 1:2]
                # (tensor_tensor_reduce crashes the exec unit on this HW;
                # use explicit multiply + reduce instead)
                sel3 = sel.rearrange("p (a b) -> p a b", b=S)
                junk3 = junk.rearrange("p (a b) -> p a b", b=S)
                nc.vector.tensor_tensor(out=junk3, in0=sel3,
                                        in1=gvc[:, :, 0:S],
                                        op=mybir.AluOpType.mult)
                nc.vector.tensor_reduce(out=num, in_=junk,
                                        axis=mybir.AxisListType.X,
                                        op=mybir.AluOpType.add)
                nc.vector.tensor_tensor(out=junk3, in0=sel3,
                                        in1=gvc[:, :, S:2 * S],
                                        op=mybir.AluOpType.mult)
                nc.vector.tensor_reduce(out=den, in_=junk,
                                        axis=mybir.AxisListType.X,
                                        op=mybir.AluOpType.add)
                den0, denp, recip = acc[:, 2:3], acc[:, 3:4], acc[:, 4:5]
                nc.vector.tensor_scalar(
                    out=den0, in0=den, scalar1=0.0, scalar2=None,
                    op0=mybir.AluOpType.is_equal)
                nc.vector.tensor_add(denp, den, den0)
                nc.vector.reciprocal(recip, denp)
                nc.vector.tensor_mul(res_sb[:, rt:rt + 1], num, recip)

                # flags: 16th==17th candidate value, or coverage ambiguous
                fa, fb = acc[:, 5:6], acc[:, 6:7]
                nc.vector.tensor_tensor(
                    out=fa, in0=v_c[:, 1, 7:8], in1=v_c[:, 2, 0:1],
                    op=mybir.AluOpType.is_equal)
                nc.vector.tensor_tensor(
                    out=fb, in0=v_seg[:, 3, 0:1], in1=v_c[:, 1, 7:8],
                    op=mybir.AluOpType.is_ge)
                nc.vector.tensor_tensor(
                    out=flag_sb[:, rt:rt + 1], in0=fa, in1=fb,
                    op=mybir.AluOpType.logical_or)

            nc.sync.dma_start(out=out_res[:, :], in_=res_sb)
            nc.sync.dma_start(out=out_flag[:, :], in_=flag_sb)

    nc.compile()
    return nc


def _host_reference_rows(dist_rows: np.ndarray, fit: np.ndarray,
                         mask: np.ndarray, k: int) -> np.ndarray:
    """Exact recompute (jax.lax.top_k tie semantics) for flagged rows."""
    out = np.empty(dist_rows.shape[0], dtype=np.float32)
    valid = (1 - mask).astype(np.float32)
    for i, row in enumerate(dist_rows):
        r = np.nan_to_num(row, nan=1e10)
        idx = np.argsort(r, kind="stable")[:k]
        w = valid[idx]
        ws = np.float32(w.sum(dtype=np.float32))
        div = ws if ws != 0 else np.float32(1.0)
        num = np.float32((fit[idx].astype(np.float32) * w).sum(dtype=np.float32))
        out[i] = num / div
    return out


def _prep_tables(fit_X_col: np.ndarray, mask_fit_X_col: np.ndarray):
    valid = (1 - mask_fit_X_col).astype(np.float32)
    g = fit_X_col.astype(np.float32) * valid
    gv_tab = np.empty((NSEG, 2, S), dtype=np.float32)
    gv_tab[:, 0, :] = g.reshape(NSEG, S)
    gv_tab[:, 1, :] = valid.reshape(NSEG, S)
    return gv_tab.reshape(NSEG, 2 * S)


def kernel(dist_pot_donors, n_neighbors, fit_X_col, mask_fit_X_col,
           _trace=False, _tmpdir=None):
    dist = np.ascontiguousarray(np.asarray(dist_pot_donors, dtype=np.float32))
    fit = np.asarray(fit_X_col, dtype=np.float32)
    mask = np.asarray(mask_fit_X_col)
    k = int(np.asarray(n_neighbors))
    assert dist.shape == (R_TOTAL, N) and k == 16, (dist.shape, k)

    gv_tab = _prep_tables(fit, mask)
    rows = R_TOTAL // N_CORES
    nt = rows // P

    nc = build_bass(rows)
    in_maps = [
        {"dist": dist[c * rows:(c + 1) * rows], "gv": gv_tab}
        for c in range(N_CORES)
    ]
    kw = {}
    if _trace:
        kw.update(trace=True, tmpdir=_tmpdir)
    br = run_bass_kernel_spmd(nc, in_maps, core_ids=list(range(N_CORES)), **kw)

    out = np.empty(R_TOTAL, dtype=np.float32)
    flags = np.empty(R_TOTAL, dtype=bool)
    for c, r in enumerate(br.results):
        # res[p, t] holds row c*rows + t*128 + p
        out[c * rows:(c + 1) * rows] = r["res"].T.reshape(rows)
        flags[c * rows:(c + 1) * rows] = r["flag"].T.reshape(rows) != 0

    n_flagged = int(flags.sum())
    if n_flagged:
        out[flags] = _host_reference_rows(dist[flags], fit, mask, k)
    kernel._last = {"exec_time_ns": br.exec_time_ns,
                    "mean_exec_time_ns": br.mean_exec_time_ns,
                    "n_flagged": n_flagged,
                    "trace": br.instructions_and_trace}
    return out
